# revision 1
# baseline (speedup 1.0000x reference)
"""Self-contained Trainium2 Bass kernel for nn_CompositionalEmbedding.

Computes, for each of N=131072 rows:
  - gather 4 embedding tables + value projection -> comp [5, 128], masked
  - 5-token multi-head attention (4 heads, head dim 32)
  - mean over tokens, project 128 -> 768

Strategy (per core, data-parallel over 8 cores, 16384 rows each):
  Rows live on the SBUF free dimension ("transposed" layout [feature, row]).
  All gathers become one-hot matmuls on the TensorEngine with host-fused
  tables (embedding @ W_qkv). The per-row mask and bias are folded into an
  augmented state index idx = (id+1)*mask (state 0 == masked-out row).
  Scores: P_qk = Q_q * K_k elementwise (DVE/GPSIMD), then per-head partition
  reduction via a 0/1 mask matmul accumulated in PSUM -> s [100, nt].
  Softmax + attention application stay as narrow matmuls + elementwise ops.
  Final projection uses W2 = (out_w @ out_proj_w)/5 fused on host.
"""

import sys
if '/opt/trn_rl_repo' not in sys.path:
    sys.path.insert(0, '/opt/trn_rl_repo')

import numpy as np

import concourse.bass as bass
import concourse.tile as tile
from concourse import bacc, mybir
from concourse.bass_utils import run_bass_kernel_spmd

F32 = mybir.dt.float32
F32R = mybir.dt.float32r
AF = mybir.ActivationFunctionType
ALU = mybir.AluOpType

N = 131072
NCORES = 8
E = 128
H = 4
DH = 32
OUT = 768
NT = 512                      # rows per tile

# token order: 0=dev,1=pseudo,2=attr,3=value,4=unit
SIZES = {0: 10, 1: 10, 2: 100, 4: 20}
SP = {j: s + 1 for j, s in SIZES.items()}   # +1 for the masked (zero) state
DISC = [0, 1, 2, 4]           # discrete tokens
TOK = [0, 1, 2, 3, 4]

# how many of the 25 score products run on GPSIMD (rest on DVE)
N_GPSIMD_MULS = 10


def _host_tables(inputs):
    """Precompute all constant tables from the (weight) inputs. O(weights)."""
    t = {}
    inv = 1.0 / np.sqrt(DH)
    Wq, Wk, Wv = (inputs['in_proj_w'][i * E:(i + 1) * E].astype(np.float64)
                  for i in range(3))
    bq, bk, bv = (inputs['in_proj_b'][i * E:(i + 1) * E].astype(np.float64)
                  for i in range(3))
    tables = {0: inputs['dev_table'], 1: inputs['pseudo_table'],
              2: inputs['attr_table'], 4: inputs['unit_table']}
    base = {0: 0, 1: 32, 2: 0, 4: 64}
    for j in DISC:
        C = np.zeros((SP[j], E))
        C[1:] = tables[j].astype(np.float64)
        qkv = [(C @ Wq.T + bq) * inv, C @ Wk.T + bk, C @ Wv.T + bv]
        for kind, arr in zip('qkv', qkv):
            pad = np.zeros((base[j] + SP[j], E))
            pad[base[j]:] = arr
            t[f'{kind}{j}'] = pad.astype(np.float32)
    # value token: comp_3 = t'*w + m3*b  (t' = m3*values), plus constant bias
    w = inputs['val_w'][:, 0].astype(np.float64)
    b = inputs['val_b'].astype(np.float64)
    t['q3'] = (np.stack([Wq @ w, Wq @ b, bq]) * inv).astype(np.float32)
    t['k3'] = np.stack([Wk @ w, Wk @ b, bk]).astype(np.float32)
    t['v3'] = np.stack([Wv @ w, Wv @ b, bv]).astype(np.float32)

    # final fused projection [768,128] including the 1/5 token mean
    W2 = (inputs['out_w'].astype(np.float64)
          @ inputs['out_proj_w'].astype(np.float64)) / 5.0
    t['w2t'] = W2.T.astype(np.float32).copy()          # [128, 768]
    c2 = (inputs['out_w'].astype(np.float64) @ inputs['out_proj_b'].astype(np.float64)
          + inputs['out_b'].astype(np.float64))
    t['c2'] = c2.astype(np.float32)                    # [768]

    # one-hot compare constants (stacked layout: d@0, p@32, u@64)
    iota = np.full((128, 1), -1.0, dtype=np.float32)
    iota[0:11, 0] = np.arange(11)
    iota[32:43, 0] = np.arange(11)
    iota[64:85, 0] = np.arange(21)
    t['iotas'] = iota
    t['iotaa'] = np.arange(101, dtype=np.float32).reshape(101, 1).copy()
    ones3 = np.zeros((3, 128), dtype=np.float32)
    ones3[0, 0:11] = 1.0
    ones3[1, 32:43] = 1.0
    ones3[2, 64:85] = 1.0
    t['ones3'] = ones3
    t['onesa'] = np.ones((1, 101), dtype=np.float32)

    # score reduce masks: s row layout r = h*25 + q*5 + k
    red = np.zeros((25, E, 100), dtype=np.float32)
    for q in range(5):
        for k in range(5):
            p = q * 5 + k
            for h in range(H):
                red[p, h * DH:(h + 1) * DH, h * 25 + q * 5 + k] = 1.0
    t['redmask'] = red.transpose(1, 0, 2).reshape(E, 2500).copy()  # [128, 25*100]

    zm = np.zeros((100, 20), dtype=np.float32)    # Z[(h,q)] = sum_k s[h,q,k]
    rm = np.zeros((20, 100), dtype=np.float32)    # broadcast r back over k
    am = np.zeros((100, 20), dtype=np.float32)    # a[(k,h)] = sum_q attn
    for h in range(H):
        for q in range(5):
            for k in range(5):
                r = h * 25 + q * 5 + k
                zm[r, h * 5 + q] = 1.0
                rm[h * 5 + q, r] = 1.0
                am[r, k * 4 + h] = 1.0
    t['zmask'] = zm
    t['rmask'] = rm
    t['akmask'] = am

    ab = np.zeros((20, 5 * E), dtype=np.float32)  # a[(k,h)] -> [E] per token k
    for k in range(5):
        for h in range(H):
            ab[k * 4 + h, k * E + h * DH:k * E + (h + 1) * DH] = 1.0
    t['abmask'] = ab
    return t


CONST_SPECS = [
    ('q0', (11, E)), ('k0', (11, E)), ('v0', (11, E)),
    ('q1', (43, E)), ('k1', (43, E)), ('v1', (43, E)),
    ('q2', (101, E)), ('k2', (101, E)), ('v2', (101, E)),
    ('q4', (85, E)), ('k4', (85, E)), ('v4', (85, E)),
    ('q3', (3, E)), ('k3', (3, E)), ('v3', (3, E)),
    ('w2t', (E, OUT)),
    ('ones3', (3, 128)), ('onesa', (1, 101)),
    ('redmask', (E, 2500)),
    ('zmask', (100, 20)), ('rmask', (20, 100)), ('akmask', (100, 20)),
    ('abmask', (20, 5 * E)),
]
F32_CONSTS = {'iotas': (128, 1), 'iotaa': (101, 1)}


def build_module(nrows):
    """Build the per-core Bass module. nrows = rows per core."""
    ntiles = nrows // NT
    nc = bacc.Bacc("TRN2", target_bir_lowering=False, debug=False,
                   num_devices=NCORES)

    dram = {}
    for name, shape in CONST_SPECS:
        dram[name] = nc.dram_tensor(name, list(shape), F32, kind="ExternalInput")
    for name, shape in F32_CONSTS.items():
        dram[name] = nc.dram_tensor(name, list(shape), F32, kind="ExternalInput")
    for name in ['idd', 'idp', 'ida', 'idu', 'm0', 'm1', 'm2', 'm3', 'm4', 'vals']:
        dram[name] = nc.dram_tensor(name, [ntiles, NT], F32, kind="ExternalInput")
    out_d = nc.dram_tensor("out", [nrows, OUT], F32, kind="ExternalOutput")

    with tile.TileContext(nc) as tc:
        with (
            tc.tile_pool(name="const", bufs=1) as cpool,
            tc.tile_pool(name="prep", bufs=1) as prep,
            tc.tile_pool(name="oh", bufs=2) as ohp,
            tc.tile_pool(name="qkv", bufs=2) as qkvp,
            tc.tile_pool(name="pp", bufs=4) as ppool,
            tc.tile_pool(name="soft", bufs=2) as softp,
            tc.tile_pool(name="outsb", bufs=2) as outsb,
            tc.tile_pool(name="ps_mm", bufs=3, space="PSUM") as ps_mm,
            tc.tile_pool(name="ps_s", bufs=1, space="PSUM") as ps_s,
            tc.tile_pool(name="ps_aux", bufs=1, space="PSUM") as ps_aux,
            tc.tile_pool(name="ps_fin", bufs=2, space="PSUM") as ps_fin,
            tc.tile_pool(name="dscratch", bufs=1, space="DRAM") as dpool,
            tc.tile_pool(name="rhs", bufs=3) as rhsp,
        ):
            # ---- constants into SBUF (cast to f32r for matmul operands) ----
            C = {}
            for name, shape in CONST_SPECS:
                tl = cpool.tile(list(shape), F32R, tag=name)
                nc.gpsimd.dma_start(tl[:], dram[name][:])
                C[name] = tl
            for name, shape in F32_CONSTS.items():
                tl = cpool.tile(list(shape), F32, tag=name)
                nc.gpsimd.dma_start(tl[:], dram[name][:])
                C[name] = tl

            # ---- per-core row inputs ----
            I = {}
            for name in ['idd', 'idp', 'ida', 'idu', 'm0', 'm1', 'm2', 'm3',
                         'm4', 'vals']:
                tl = prep.tile([ntiles, NT], F32, tag=name)
                nc.gpsimd.dma_start(tl[:], dram[name][:])
                I[name] = tl

            # ---- index prep: idx = (id+1)*mask, tv = vals*m3 ----
            tmp = {}
            for j, (idn, mn) in {0: ('idd', 'm0'), 1: ('idp', 'm1'),
                                 2: ('ida', 'm2'), 4: ('idu', 'm4')}.items():
                a = prep.tile([ntiles, NT], F32, tag=f'ix{j}a')
                nc.vector.tensor_scalar_add(a[:], I[idn][:], 1.0)
                b = prep.tile([ntiles, NT], F32, tag=f'ix{j}b')
                nc.vector.tensor_mul(b[:], a[:], I[mn][:])
                tmp[j] = b
            tv = prep.tile([ntiles, NT], F32, tag='tv')
            nc.vector.tensor_mul(tv[:], I['vals'][:], I['m3'][:])
            onesr = prep.tile([ntiles, NT], F32, tag='ones')
            nc.vector.memset(onesr[:], 1.0)

            # ---- flatten to DRAM scratch [k, nrows]; per-tile rhs loads [k, NT] ----
            idx3d = dpool.tile([3, ntiles * NT], F32, tag='idx3d')
            vrhsd = dpool.tile([3, ntiles * NT], F32, tag='vrhsd')
            idxud = dpool.tile([1, ntiles * NT], F32, tag='idxud')
            for r, src in [(0, tmp[0]), (1, tmp[1]), (2, tmp[4])]:
                nc.gpsimd.dma_start(idx3d[r:r + 1, :], src[:])
            for r, src in [(0, tv), (1, I['m3']), (2, onesr)]:
                nc.gpsimd.dma_start(vrhsd[r:r + 1, :], src[:])
            nc.gpsimd.dma_start(idxud[0:1, :], tmp[2][:])

            # pair -> mul engine assignment (round robin-ish)
            pair_engine = {}
            cnt = 0
            for q in range(5):
                for k in range(5):
                    pair_engine[(q, k)] = (
                        nc.gpsimd if cnt < N_GPSIMD_MULS else nc.vector)
                    cnt += 1

            for t in range(ntiles):
                sl = slice(t * NT, (t + 1) * NT)
                idx3_t = rhsp.tile([3, NT], F32R, tag='idx3t')
                nc.gpsimd.dma_start(idx3_t[:], idx3d[:, sl])
                vrhs_t = rhsp.tile([3, NT], F32R, tag='vrhst')
                nc.gpsimd.dma_start(vrhs_t[:], vrhsd[:, sl])
                idxu_t = rhsp.tile([1, NT], F32R, tag='idxut')
                nc.gpsimd.dma_start(idxu_t[:], idxud[:, sl])
                # ---- one-hot construction (d@0, p@32, u@64; attr separate) ----
                bs = ps_mm.tile([128, NT], F32, tag='mm')
                nc.tensor.matmul(bs[:], C['ones3'][:], idx3_t[:],
                                 start=True, stop=True)
                Rs = ohp.tile([128, NT], F32R, tag='Rs')
                nc.vector.tensor_scalar(Rs[:], bs[:], C['iotas'][:],
                                        None, op0=ALU.is_equal)
                ba = ps_mm.tile([101, NT], F32, tag='mm')
                nc.tensor.matmul(ba[:], C['onesa'][:], idxu_t[:],
                                 start=True, stop=True)
                Ra = ohp.tile([101, NT], F32R, tag='Ra')
                nc.vector.tensor_scalar(Ra[:], ba[:], C['iotaa'][:],
                                        None, op0=ALU.is_equal)
                R = {0: Rs[0:11, :], 1: Rs[32:43, :], 4: Rs[64:85, :],
                     2: Ra[:], 3: vrhs_t[:]}

                # ---- Q, K, V builds ----
                Q, K, V = {}, {}, {}
                lhs_lo = {0: 0, 1: 32, 2: 0, 4: 64, 3: 0}
                for j in TOK:
                    for kind, store in (('q', Q), ('k', K), ('v', V)):
                        ps = ps_mm.tile([E, NT], F32, tag='mm')
                        nc.tensor.matmul(ps[:], C[f'{kind}{j}'][lhs_lo[j]:, :],
                                         R[j], start=True, stop=True)
                        sb = qkvp.tile([E, NT], F32R, tag=f'{kind}{j}')
                        nc.scalar.copy(sb[:], ps[:])
                        store[j] = sb

                # ---- scores: P_qk then mask-reduce accumulate ----
                s_ps = ps_s.tile([100, NT], F32, tag='scores')
                for q in range(5):
                    for k in range(5):
                        p = q * 5 + k
                        P = ppool.tile([E, NT], F32R, tag='P')
                        pair_engine[(q, k)].tensor_mul(P[:], Q[q][:], K[k][:])
                        nc.tensor.matmul(
                            s_ps[:], C['redmask'][:, p * 100:(p + 1) * 100],
                            P[:], start=(p == 0), stop=(p == 24))

                # ---- softmax pieces ----
                expw = softp.tile([100, NT], F32R, tag='expw')
                nc.scalar.activation(expw[:], s_ps[:], AF.Exp)
                z_ps = ps_aux.tile([100, NT], F32, tag='aux')
                nc.tensor.matmul(z_ps[:20, :], C['zmask'][:], expw[:],
                                 start=True, stop=True)
                r_sb = softp.tile([20, NT], F32R, tag='rsb')
                with nc.allow_low_precision(reason="f32r rounding for PE rhs"):
                    nc.vector.reciprocal(r_sb[:], z_ps[:20, :])
                rb_ps = ps_aux.tile([100, NT], F32, tag='aux')
                nc.tensor.matmul(rb_ps[:], C['rmask'][:], r_sb[:],
                                 start=True, stop=True)
                A = softp.tile([100, NT], F32R, tag='A')
                nc.vector.tensor_mul(A[:], expw[:], rb_ps[:])
                ak_ps = ps_aux.tile([100, NT], F32, tag='aux')
                nc.tensor.matmul(ak_ps[:20, :], C['akmask'][:], A[:],
                                 start=True, stop=True)
                ak = softp.tile([20, NT], F32R, tag='ak')
                nc.scalar.copy(ak[:], ak_ps[:20, :])

                # ---- compose: sum_k broadcast(a_k) * V_k ----
                comp = softp.tile([E, NT], F32R, tag='comp')
                for k in range(5):
                    ab_ps = ps_mm.tile([E, NT], F32, tag='mm')
                    nc.tensor.matmul(ab_ps[:],
                                     C['abmask'][:, k * E:(k + 1) * E],
                                     ak[:], start=True, stop=True)
                    if k == 0:
                        nc.vector.tensor_mul(comp[:], V[k][:], ab_ps[:])
                    else:
                        pk = ppool.tile([E, NT], F32, tag='P')
                        nc.vector.tensor_mul(pk[:], V[k][:], ab_ps[:])
                        nc.vector.tensor_add(comp[:], comp[:], pk[:])

                # ---- final projection: [NT,128] @ W2T -> [NT,768] ----
                o_sb = outsb.tile([E, 4 * OUT], F32, tag='osb')
                for c in range(4):
                    lhsT = comp[:, c * E:(c + 1) * E]
                    for half in range(2):
                        fin = ps_fin.tile([E, 384], F32, tag='fin')
                        nc.tensor.matmul(fin[:], lhsT,
                                         C['w2t'][:, half * 384:(half + 1) * 384],
                                         start=True, stop=True)
                        dst = o_sb[:, c * OUT + half * 384:c * OUT + (half + 1) * 384]
                        if c % 2 == 0:
                            nc.vector.tensor_copy(dst, fin[:])
                        else:
                            nc.scalar.copy(dst, fin[:])

                dview = out_d[t * NT:(t + 1) * NT, :].rearrange(
                    "(c p) f -> p c f", c=4)
                nc.gpsimd.dma_start(dview, o_sb[:].rearrange(
                    "p (c f) -> p c f", c=4))
    nc.compile()
    return nc


def _marshal_core(inputs, tables, c, nrows):
    """Per-core input map (host marshaling: slicing + dtype casts only)."""
    ntiles = nrows // NT
    lo = c * nrows
    sl = slice(lo, lo + nrows)
    wrap = lambda a: np.ascontiguousarray(
        np.asarray(a, dtype=np.float32).reshape(ntiles, NT))
    m = {
        'idd': wrap(inputs['device_ids'][sl]),
        'idp': wrap(inputs['pseudo_ids'][sl]),
        'ida': wrap(inputs['attr_ids'][sl]),
        'idu': wrap(inputs['unit_ids'][sl]),
        'vals': wrap(inputs['values'][sl]),
    }
    for j in range(5):
        m[f'm{j}'] = wrap(inputs['mask'][sl, j])
    for name, _ in CONST_SPECS:
        m[name] = tables[name]
    for name in F32_CONSTS:
        m[name] = tables[name]
    return m


_CACHE = {}


def _get_module(nrows):
    if nrows not in _CACHE:
        _CACHE[nrows] = build_module(nrows)
    return _CACHE[nrows]


def kernel(**inputs):
    n = inputs['device_ids'].shape[0]
    nrows = n // NCORES
    tables = _host_tables(inputs)
    nc = _get_module(nrows)
    in_maps = [_marshal_core(inputs, tables, c, nrows) for c in range(NCORES)]
    res = run_bass_kernel_spmd(nc, in_maps, core_ids=list(range(NCORES)),
                               trace=False)
    out = np.concatenate([res.results[c]['out'] for c in range(NCORES)], axis=0)
    c2 = tables['c2']
    if np.any(c2):
        out = out + c2[None, :]
    return out.astype(np.float32)


if __name__ == '__main__':
    # small smoke test with synthetic inputs
    rng = np.random.default_rng(0)
    n = 8 * int(__import__('os').environ.get('NROWS', '1024'))
    inp = {
        'device_ids': rng.integers(0, 10, n),
        'pseudo_ids': rng.integers(0, 10, n),
        'attr_ids': rng.integers(0, 100, n),
        'unit_ids': rng.integers(0, 20, n),
        'values': rng.standard_normal(n).astype(np.float32),
        'mask': rng.integers(0, 2, (n, 5)).astype(np.int32),
        'dev_table': 0.02 * rng.standard_normal((10, E)).astype(np.float32),
        'pseudo_table': 0.02 * rng.standard_normal((10, E)).astype(np.float32),
        'attr_table': 0.02 * rng.standard_normal((100, E)).astype(np.float32),
        'unit_table': 0.02 * rng.standard_normal((20, E)).astype(np.float32),
        'val_w': (rng.standard_normal((E, 1)) * (6.0 / (1 + E)) ** 0.5).astype(np.float32),
        'val_b': np.zeros(E, np.float32),
        'in_proj_w': (rng.standard_normal((3 * E, E)) / np.sqrt(E)).astype(np.float32),
        'in_proj_b': np.zeros(3 * E, np.float32),
        'out_proj_w': (rng.standard_normal((E, E)) / np.sqrt(E)).astype(np.float32),
        'out_proj_b': np.zeros(E, np.float32),
        'out_w': (rng.standard_normal((OUT, E)) * (6.0 / (E + OUT)) ** 0.5).astype(np.float32),
        'out_b': np.zeros(OUT, np.float32),
    }
    out = kernel(**inp)

    # numpy reference
    def ref(i):
        d = i['dev_table'][i['device_ids']]
        p = i['pseudo_table'][i['pseudo_ids']]
        a = i['attr_table'][i['attr_ids']]
        v = i['values'][:, None] * i['val_w'][:, 0][None, :] + i['val_b']
        u = i['unit_table'][i['unit_ids']]
        comp = np.stack([d, p, a, v, u], 1) * i['mask'][..., None]
        qkv = comp @ i['in_proj_w'].T + i['in_proj_b']
        q, k, vv = np.split(qkv, 3, -1)
        q = q.reshape(-1, 5, H, DH)
        k = k.reshape(-1, 5, H, DH)
        vv = vv.reshape(-1, 5, H, DH)
        s = np.einsum('nqhd,nkhd->nhqk', q, k) / np.sqrt(DH)
        s = s - s.max(-1, keepdims=True)
        e = np.exp(s)
        at = e / e.sum(-1, keepdims=True)
        o = np.einsum('nhqk,nkhd->nqhd', at, vv).reshape(-1, 5, E)
        o = o @ i['out_proj_w'].T + i['out_proj_b']
        return o.mean(1) @ i['out_w'].T + i['out_b']

    exp = ref(inp)
    err = np.abs(out - exp).max()
    scale = np.abs(exp).max()
    print(f"absmax err {err:.3e}  scale {scale:.3e}  rel {err/scale:.3e}")
    nr = n // 8
    pertile = np.abs(out - exp).reshape(8, nr // 512, 512, OUT).max(axis=(2, 3))
    for c in range(8):
        print('core', c, np.array2string(pertile[c], precision=1))

